# revision 13
# baseline (speedup 1.0000x reference)
"""DiT block kernel for Trainium2, data-parallel over batch across 8 NeuronCores.

Problem: nn_DiTBlock — B=8, S=1024, E=512, H=8 (head_dim = E = 512).
Sharding: batch element b -> core b. Each core runs the full DiT block on its
(S, E) slice with replicated weights; no collectives.

Strategy vs the f32r baseline (618us):
- All attention matmuls run fp8e4 with MatmulPerfMode.DoubleRow (2 contraction
  chunks of 128 packed in the free dims; ~2x the bf16/f32r PE rate). Weights
  are host-cast to fp8 (scaled x32/x64 into e4m3's normal range; the scales
  statically fold into drain ops).
- The K projection is eliminated: host precomputes M_h = wq_h @ wk_h^T (valid
  because bq/bk are structurally zero here), so scores = (y @ M_h) @ y^T with
  y^T as the stationary for every head.
- Softmax denominators come from the PE via an all-ones fp8 stationary
  (replicated across all 128 partitions), not a DVE add tree.
- V bias is folded through the linear attention average into lvb (host).
- alpha1 is applied once at the end (attn_acc accumulates raw o@lvw over
  heads), so lvw needs no per-head device-side scaling.
- FFN stays bf16 (fp8 there pushed max rel err to ~1.9e-2; bf16 FFN lands
  ~9e-3 with fp8 attention).
- PSUM tiles are [128,1024] (2 banks, 4 DoubleRow regions); each drains with
  ONE 1024-wide op into a contiguous fp8 plane pair, halving drain-op count.
  ACT takes exp + V, DVE takes A/oT/acc/recip. start=True zeroes a whole
  2KB bank, so only each bank's first matmul sets it.
"""
import sys
import numpy as np

sys.path.insert(0, '/opt/trn_rl_repo')

B, S, E, H = 8, 1024, 512, 8
HE = H * E          # 4096
FF = 4 * E          # 2048
EPS = 1e-5
SCALE = 1.0 / 32.0  # 1/sqrt(S)
WS = 32.0           # static fp8 scale for wv, lvw
MS = 64.0           # static fp8 scale for M_h = wq_h @ wk_h^T
N_CORES = 8

TRACE = False       # set by test harness to capture an NTFF profile
TRACE_DIR = None

_CACHE = {}


def _build():
    from contextlib import ExitStack
    import concourse.bass as bass
    import concourse.tile as tile
    from concourse import bacc, mybir
    f32 = mybir.dt.float32
    bf16 = mybir.dt.bfloat16
    fp8 = mybir.dt.float8e4
    AF = mybir.ActivationFunctionType
    ALU = mybir.AluOpType
    DR = mybir.MatmulPerfMode.DoubleRow

    nc = bacc.Bacc("TRN2", target_bir_lowering=False, debug=False,
                   num_devices=N_CORES)

    # ---- DRAM parameters --------------------------------------------------
    x_d = nc.dram_tensor("x", [S, E], f32, kind="ExternalInput").ap()
    cond_d = nc.dram_tensor("cond", [E, 1], f32, kind="ExternalInput").ap()

    adaln_w = {}
    adaln_b = {}
    for nm in ["g1", "be1", "a1", "g2", "be2", "a2"]:
        adaln_w[nm] = nc.dram_tensor(f"{nm}w", [E, E], f32,
                                     kind="ExternalInput").ap()
        adaln_b[nm] = nc.dram_tensor(f"{nm}b", [1, E], f32,
                                     kind="ExternalInput").ap()
    ln1g_d = nc.dram_tensor("ln1g", [1, E], f32, kind="ExternalInput").ap()
    ln1b_d = nc.dram_tensor("ln1b", [1, E], f32, kind="ExternalInput").ap()
    ln2g_d = nc.dram_tensor("ln2g", [1, E], f32, kind="ExternalInput").ap()
    ln2b_d = nc.dram_tensor("ln2b", [1, E], f32, kind="ExternalInput").ap()
    # fp8 paired-layout attention weights (host-prepared; see kernel())
    m8_d = nc.dram_tensor("m8", [H, 2, 128, 2, E], fp8,
                          kind="ExternalInput").ap()
    wv8_d = nc.dram_tensor("wv8", [H, 2, 128, 2, E], fp8,
                           kind="ExternalInput").ap()
    lvw8_d = nc.dram_tensor("lvw8", [H, 2, 128, 2, E], fp8,
                            kind="ExternalInput").ap()
    # bf16 FFN weights
    f1w_d = nc.dram_tensor("f1w16", [E, FF], bf16, kind="ExternalInput").ap()
    f2w_d = nc.dram_tensor("f2w16", [FF, E], bf16, kind="ExternalInput").ap()
    lvb_d = nc.dram_tensor("lvb", [1, E], f32, kind="ExternalInput").ap()
    f1b_d = nc.dram_tensor("f1b", [1, FF], f32, kind="ExternalInput").ap()
    f2b_d = nc.dram_tensor("f2b", [1, E], f32, kind="ExternalInput").ap()
    ident_d = nc.dram_tensor("ident", [128, 128], bf16,
                             kind="ExternalInput").ap()
    out_d = nc.dram_tensor("out", [S, E], f32, kind="ExternalOutput").ap()

    with tile.TileContext(nc) as tc, ExitStack() as ctx:
        const = ctx.enter_context(tc.tile_pool(name="const", bufs=1))
        work = ctx.enter_context(tc.tile_pool(name="work", bufs=3))
        psum_mm = ctx.enter_context(
            tc.tile_pool(name="psum_mm", bufs=3, space="PSUM"))
        psum_tp = ctx.enter_context(
            tc.tile_pool(name="psum_tp", bufs=1, space="PSUM"))
        psum_row = ctx.enter_context(
            tc.tile_pool(name="psum_row", bufs=1, space="PSUM"))

        # ---- constants ----
        ident = const.tile([128, 128], bf16)
        nc.sync.dma_start(ident, ident_d)
        eps_t = const.tile([128, 1], f32)
        nc.vector.memset(eps_t, EPS)
        ones8 = const.tile([128, 2, 128], fp8)
        nc.vector.memset(ones8, 1.0)

        cond_cols = const.tile([128, 4], f32)
        nc.sync.dma_start(cond_cols, cond_d.rearrange("(c p) o -> p (c o)", p=128))

        adp = ctx.enter_context(tc.tile_pool(name="adp", bufs=2))

        def adaln_cols(nm):
            """(cond @ W + b) laid out as [128, 4] e-columns (PE matvec)."""
            pcol = psum_row.tile([128, 4], f32, name=f"pcol_{nm}", tag="prow")
            for kc in range(4):
                adw = adp.tile([128, E], f32, name=f"adw_{nm}_{kc}", tag="adw")
                nc.sync.dma_start(adw, adaln_w[nm][kc * 128:(kc + 1) * 128, :])
                for ec in range(4):
                    nc.tensor.matmul(
                        pcol[:, ec:ec + 1],
                        adw[:, ec * 128:(ec + 1) * 128],
                        cond_cols[:, kc:kc + 1],
                        start=(kc == 0 and ec == 0),
                        stop=(kc == 3 and ec == 3))
            bcol = adp.tile([128, 4], f32, name=f"bcol_{nm}", tag="bcol")
            nc.sync.dma_start(
                bcol, adaln_b[nm].rearrange("o (c p) -> (o p) c", p=128))
            mcol = const.tile([128, 4], f32, name=f"mcol_{nm}")
            nc.vector.tensor_add(mcol, pcol, bcol)
            return mcol

        def adaln_rep2(nm, post_scale=None):
            """(cond @ W + b) replicated to [128, 2E] (both halves)."""
            prow = psum_row.tile([1, E], f32, name=f"prow_{nm}", tag="prow")
            for kc in range(4):
                adw = adp.tile([128, E], f32, name=f"adw_{nm}_{kc}", tag="adw")
                nc.sync.dma_start(adw, adaln_w[nm][kc * 128:(kc + 1) * 128, :])
                nc.tensor.matmul(prow, cond_cols[:, kc:kc + 1], adw,
                                 start=(kc == 0), stop=(kc == 3))
            brow = adp.tile([1, E], f32, name=f"brow_{nm}", tag="brow")
            nc.sync.dma_start(brow, adaln_b[nm])
            arow = adp.tile([1, E], f32, name=f"arow_{nm}", tag="arow")
            nc.vector.tensor_add(arow, prow, brow)
            arep = const.tile([128, E], f32, name=f"arep_{nm}")
            nc.gpsimd.partition_broadcast(arep, arow)
            a2 = const.tile([128, 2 * E], f32, name=f"a2_{nm}")
            for half in range(2):
                if post_scale is None:
                    nc.vector.tensor_copy(a2[:, half * E:(half + 1) * E], arep)
                else:
                    nc.vector.tensor_scalar(
                        a2[:, half * E:(half + 1) * E], arep,
                        scalar1=post_scale, scalar2=None, op0=ALU.mult)
            return a2

        def scale_shift(li, gcol, bcol, lng_d, lnb_d):
            """sc = ln_g*(1+gamma), bi = ln_b*(1+gamma)+beta, as [128,4] cols."""
            gp = const.tile([128, 4], f32, name=f"gp_{li}")
            nc.vector.tensor_scalar_add(gp, gcol, 1.0)
            lgc = adp.tile([128, 4], f32, name=f"lgc_{li}", tag="lgc")
            nc.sync.dma_start(lgc, lng_d.rearrange("o (c p) -> (o p) c", p=128))
            lbc = adp.tile([128, 4], f32, name=f"lbc_{li}", tag="lbc")
            nc.sync.dma_start(lbc, lnb_d.rearrange("o (c p) -> (o p) c", p=128))
            sc = const.tile([128, 4], f32, name=f"sc_{li}")
            nc.vector.tensor_mul(sc, lgc, gp)
            bi = const.tile([128, 4], f32, name=f"bi_{li}")
            nc.vector.tensor_mul(bi, lbc, gp)
            nc.vector.tensor_add(bi, bi, bcol)
            return sc, bi

        # Only g1/be1 gate the LN1->y^T critical path; defer the rest.
        sc1, bi1 = scale_shift(0, adaln_cols("g1"), adaln_cols("be1"),
                               ln1g_d, ln1b_d)

        # ---- persistent activation tiles ----
        # yT8[p][:, j, s] = y^T[e, s] for e-chunk 2p+j (fp8, unscaled)
        yT8 = [const.tile([128, 2, S], fp8, name=f"yT8_{p}") for p in range(2)]
        # t-pair-merged accumulators: [128, 1024] = tiles (2u, 2u+1)
        attn_acc = [const.tile([128, 2 * E], f32, name=f"acc{u}")
                    for u in range(4)]
        y2 = [const.tile([128, 2 * E], f32, name=f"y2_{u}") for u in range(4)]
        zT16 = [const.tile([128, S], bf16, name=f"zT{c}") for c in range(4)]

        def ln_stats(x_t, tagp):
            st = work.tile([128, 6], f32, name=f"st_{tagp}", tag=f"st_{tagp}")
            nc.vector.bn_stats(st, x_t)
            mv = work.tile([128, 2], f32, name=f"mv_{tagp}", tag=f"mv_{tagp}")
            nc.vector.bn_aggr(mv, st)
            rs = work.tile([128, 1], f32, name=f"rs_{tagp}", tag=f"rs_{tagp}")
            nc.scalar.activation(rs, mv[:, 1:2], AF.Sqrt, bias=eps_t, scale=1.0)
            nc.vector.reciprocal(rs, rs)
            xn = work.tile([128, E], bf16, name=f"xn_{tagp}", tag="wxn")
            nc.vector.tensor_scalar(xn, x_t, scalar1=mv[:, 0:1], scalar2=rs,
                                    op0=ALU.subtract, op1=ALU.mult)
            return xn

        def ln_transpose_pair(xns, scol, bcol, dst_cb, tagp):
            """PE-transpose a pair of normalized tiles; fused modulate ACT."""
            for ec in range(4):
                tp = psum_tp.tile([128, 256], bf16, name=f"tp_{tagp}",
                                  tag="ptp")
                nc.tensor.matmul(tp[:, 0:128],
                                 xns[0][:, ec * 128:(ec + 1) * 128], ident,
                                 is_transpose=True, start=True, stop=False)
                nc.tensor.matmul(tp[:, 128:256],
                                 xns[1][:, ec * 128:(ec + 1) * 128], ident,
                                 is_transpose=True, start=False, stop=True)
                nc.scalar.activation(
                    dst_cb(ec), tp, AF.Identity,
                    bias=bcol[:, ec:ec + 1], scale=scol[:, ec:ec + 1])

        # FFN weights prefetched in chunks during heads 4-7.
        ffp = ctx.enter_context(tc.tile_pool(name="ffp", bufs=1))
        f1w_t = [ffp.tile([128, FF], bf16, name=f"f1w{kc}", tag=f"f1w{kc}")
                 for kc in range(4)]
        f1bc = ffp.tile([128, 16], f32, tag="f1bc")
        f2w_t = [ffp.tile([128, E], bf16, name=f"f2w{kc}", tag=f"f2w{kc}")
                 for kc in range(16)]

        def emit_ffw_quarter(q):
            for kc in range(4):
                nc.sync.dma_start(
                    f1w_t[kc][:, q * 512:(q + 1) * 512],
                    f1w_d[kc * 128:(kc + 1) * 128, q * 512:(q + 1) * 512])
            for kc in range(4):
                nc.sync.dma_start(
                    f2w_t[4 * q + kc],
                    f2w_d[(4 * q + kc) * 128:(4 * q + kc + 1) * 128, :])
            if q == 3:
                nc.sync.dma_start(
                    f1bc, f1b_d.rearrange("o (c p) -> (o p) c", p=128))

        hp_ctx = ExitStack()
        hp = hp_ctx.enter_context(tc.tile_pool(name="hp", bufs=1))

        def emit_head_dma(h):
            """Load M, wv, lvw fp8 pair tiles for head h."""
            m_t = [hp.tile([128, 2, E], fp8, name=f"m{p}", tag=f"m{p}_{h % 2}")
                   for p in range(2)]
            wv_t = [hp.tile([128, 2, E], fp8, name=f"wv{p}",
                            tag=f"wv{p}_{h % 2}") for p in range(2)]
            lvw_t = [hp.tile([128, 2, E], fp8, name=f"lvw{p}",
                             tag=f"lvw{p}_{h % 2}") for p in range(2)]
            for p in range(2):
                nc.gpsimd.dma_start(m_t[p], m8_d[h, p])
                nc.sync.dma_start(wv_t[p], wv8_d[h, p])
                nc.sync.dma_start(lvw_t[p], lvw8_d[h, p])
            return m_t, wv_t, lvw_t

        MOD = {}

        def emit_adaln_a1_seed():
            # y2 seed = x + lvb_eff*a1; attention branch lands at the end:
            # y2 += attn_acc * (a1/1024)
            MOD["A1c2"] = adaln_rep2("a1", post_scale=1.0 / (WS * WS))
            A12 = MOD["A1c2"]
            # lvb arrives host-scaled by 1024, so LVBA2 = lvb1024 * (a1/1024)
            # = lvb_eff * a1 in one op.
            LVBA2 = const.tile([128, 2 * E], f32)
            for half in range(2):
                nc.sync.dma_start(LVBA2[:, half * E:(half + 1) * E],
                                  lvb_d.broadcast_to([128, E]))
            nc.gpsimd.tensor_tensor(LVBA2, LVBA2, A12, op=ALU.mult)
            for u in range(4):
                x_t3 = work.tile([128, 2 * E], f32, name="xt3", tag="wbig")
                for half in range(2):
                    t = 2 * u + half
                    nc.sync.dma_start(x_t3[:, half * E:(half + 1) * E],
                                      x_d[t * 128:(t + 1) * 128, :])
                nc.vector.tensor_add(y2[u], x_t3, LVBA2)

        def emit_adaln_ln2():
            MOD["sc2"], MOD["bi2"] = scale_shift(
                1, adaln_cols("g2"), adaln_cols("be2"), ln2g_d, ln2b_d)
            MOD["A22"] = adaln_rep2("a2")
            F2B2 = const.tile([128, 2 * E], f32)
            for half in range(2):
                nc.sync.dma_start(F2B2[:, half * E:(half + 1) * E],
                                  f2b_d.broadcast_to([128, E]))
            MOD["F2B2"] = F2B2

        def ln2_pair(u):
            """Finalize y2 pair u and emit its LN2 transpose into zT16."""
            t1 = work.tile([128, 2 * E], f32, name="t1", tag="wbig")
            nc.vector.tensor_mul(t1, attn_acc[u], MOD["A1c2"])
            nc.vector.tensor_add(y2[u], y2[u], t1)
            xns = [ln_stats(y2[u][:, half * E:(half + 1) * E], "ln2")
                   for half in range(2)]
            ln_transpose_pair(
                xns, MOD["sc2"], MOD["bi2"],
                lambda ec: zT16[ec][:, 2 * u * 128:(2 * u + 2) * 128], "ln2")

        # ---- Phase 1: LN1 -> y^T (fp8 paired planes) ----
        PRE0 = emit_head_dma(0)
        for t0 in range(0, 8, 2):
            xns = []
            for t in (t0, t0 + 1):
                x_t = work.tile([128, E], f32, name="xt_ln1", tag="wbig")
                nc.sync.dma_start(x_t, x_d[t * 128:(t + 1) * 128, :])
                xns.append(ln_stats(x_t, "ln1"))
            ln_transpose_pair(
                xns, sc1, bi1,
                lambda ec: yT8[ec // 2][:, ec % 2, t0 * 128:(t0 + 2) * 128],
                "ln1")

        # ---- Phase 2: attention heads (software-pipelined) ----
        # A [128,1024] psum tile spans 2 banks = 4 DoubleRow regions; start
        # only on each bank's first matmul (start zeroes the whole bank).
        def dr_accum(ps, stat_fn, mov_fn, npass):
            for reg in range(4):
                for q in range(npass):
                    nc.tensor.matmul(
                        ps[:, reg * 256:(reg + 1) * 256],
                        stat_fn(q), mov_fn(q, reg),
                        start=(q == 0 and reg % 2 == 0), stop=(q == npass - 1),
                        perf_mode=DR)

        def head_A(h, m_t):
            """A^T = (y @ M_h*64)^T, fp8 paired planes. Drain: DVE copy."""
            AT8 = [hp.tile([128, 2, S], fp8, name=f"AT{p}",
                           tag=f"AT{p}_{h % 2}") for p in range(2)]
            for mc in range(4):
                pa = psum_mm.tile([128, 1024], f32, name="pa", tag="pmm")
                dr_accum(
                    pa,
                    lambda p: m_t[p][:, :, mc * 128:(mc + 1) * 128],
                    lambda p, n4: yT8[p][:, :, n4 * 256:(n4 + 1) * 256], 2)
                nc.vector.tensor_copy(AT8[mc // 2][:, mc % 2, :], pa)
            return AT8

        def head_v(h, wv_t):
            """V_h*32 (natural layout), fp8 paired planes. Drain: ACT."""
            Vh8 = [hp.tile([128, 2, E], fp8, name=f"V{tp}",
                           tag=f"V{tp}_{h % 2}") for tp in range(4)]
            for u in range(4):
                pv = psum_mm.tile([128, 1024], f32, name="pv", tag="pmm")
                for half in range(2):
                    for c in range(2):
                        for p in range(2):
                            nc.tensor.matmul(
                                pv[:, half * 512 + c * 256:
                                   half * 512 + (c + 1) * 256],
                                yT8[p][:, :,
                                       (2 * u + half) * 128:
                                       (2 * u + half + 1) * 128],
                                wv_t[p][:, :, c * 256:(c + 1) * 256],
                                start=(c == 0 and p == 0), stop=(p == 1),
                                perf_mode=DR)
                nc.scalar.activation(Vh8[u], pv, AF.Identity, scale=1.0)
            return Vh8

        def head_scores(h, AT8):
            """scores^T = y A^T (contract head dim); exp via ACT -> fp8."""
            Eh8 = [hp.tile([128, 2, S], fp8, name=f"E{tp}",
                           tag=f"E{tp}_{h % 2}") for tp in range(4)]
            for t in range(8):
                ps = psum_mm.tile([128, 1024], f32, name="ps", tag="pmm")
                dr_accum(
                    ps,
                    lambda p: yT8[p][:, :, t * 128:(t + 1) * 128],
                    lambda p, n4: AT8[p][:, :, n4 * 256:(n4 + 1) * 256], 2)
                nc.scalar.activation(Eh8[t // 2][:, t % 2, :], ps,
                                     AF.Exp, scale=SCALE / MS)
            return Eh8

        def head_sums(h, Eh8):
            """Softmax denominators replicated across partitions (all-ones
            fp8 stationary); reciprocal on DVE."""
            Rrep = hp.tile([128, S], f32, tag=f"rrep_{h % 2}")
            pr = psum_mm.tile([128, 1024], f32, name="pr", tag="pmm")
            dr_accum(
                pr,
                lambda tp: ones8,
                lambda tp, n4: Eh8[tp][:, :, n4 * 256:(n4 + 1) * 256], 4)
            nc.vector.reciprocal_approx_fast(Rrep, pr)
            return Rrep

        def head_av(h, Vh8, Eh8, Rrep):
            """o^T*32 = (V*32)^T E^T / sums, fp8. Drain: DVE TT mult."""
            oT8 = [hp.tile([128, 2, S], fp8, name=f"oT{p}",
                           tag=f"oT{p}_{h % 2}") for p in range(2)]
            for ec in range(4):
                po = psum_mm.tile([128, 1024], f32, name="po", tag="pmm")
                dr_accum(
                    po,
                    lambda tp: Vh8[tp][:, :, ec * 128:(ec + 1) * 128],
                    lambda tp, n4: Eh8[tp][:, :, n4 * 256:(n4 + 1) * 256], 4)
                nc.vector.tensor_tensor(oT8[ec // 2][:, ec % 2, :], po, Rrep,
                                        op=ALU.mult)
            return oT8

        def head_lv(h, oT8, lvw_t, last):
            """attn_acc += (o^T*32)^T @ (lvw*32); scale folded at the end."""
            for u in range(4):
                pl = psum_mm.tile([128, 1024], f32, name="pl", tag="pmm")
                for half in range(2):
                    for c in range(2):
                        for p in range(2):
                            nc.tensor.matmul(
                                pl[:, half * 512 + c * 256:
                                   half * 512 + (c + 1) * 256],
                                oT8[p][:, :,
                                       (2 * u + half) * 128:
                                       (2 * u + half + 1) * 128],
                                lvw_t[p][:, :, c * 256:(c + 1) * 256],
                                start=(c == 0 and p == 0), stop=(p == 1),
                                perf_mode=DR)
                if h == 0:
                    nc.vector.tensor_copy(attn_acc[u], pl)
                else:
                    nc.vector.tensor_add(attn_acc[u], attn_acc[u], pl)
                if last:
                    ln2_pair(u)

        # Pipelined loop. PE stream per iteration:
        #   A(h) -> V(h) -> scores(h) -> sums(h-1) -> AV(h-1) -> lv(h-1)
        prev = None
        for h in range(H):
            m_t, wv_t, lvw_t = PRE0 if h == 0 else emit_head_dma(h)
            AT8 = head_A(h, m_t)
            Vh8 = head_v(h, wv_t)
            Eh8 = head_scores(h, AT8)
            if prev is not None:
                ph, pV, pE, plvw = prev
                Rrep = head_sums(ph, pE)
                oT8 = head_av(ph, pV, pE, Rrep)
                head_lv(ph, oT8, plvw, last=False)
            if h == 0:
                emit_adaln_a1_seed()
            elif h == 2:
                emit_adaln_ln2()
            elif h >= 4:
                emit_ffw_quarter(h - 4)
            prev = (h, Vh8, Eh8, lvw_t)
        ph, pV, pE, plvw = prev
        Rrep = head_sums(ph, pE)
        oT8 = head_av(ph, pV, pE, Rrep)
        head_lv(ph, oT8, plvw, last=True)
        hp_ctx.close()

        # keep the PE clock-gate open across the LN2 boundary
        fp = ctx.enter_context(tc.tile_pool(name="fp", bufs=1))
        for i in range(12):
            warm2 = psum_tp.tile([128, 256], bf16, name="warm2", tag="ptp")
            nc.tensor.matmul(warm2[:, 0:128], ident, ident,
                             start=True, stop=True, is_transpose=True)

        # ---- Phase 5: FFN (bf16) ----
        hT = [fp.tile([128, S], bf16, name=f"hT{hc}", tag=f"hT{hc}")
              for hc in range(16)]
        for hc in range(16):
            pf = psum_mm.tile([128, 1024], f32, name="pf", tag="pmm")
            for sh in range(2):
                for kc in range(4):
                    nc.tensor.matmul(
                        pf[:, sh * 512:(sh + 1) * 512],
                        f1w_t[kc][:, hc * 128:(hc + 1) * 128],
                        zT16[kc][:, sh * 512:(sh + 1) * 512],
                        start=(kc == 0), stop=(kc == 3))
            nc.scalar.activation(hT[hc], pf, AF.Relu,
                                 bias=f1bc[:, hc:hc + 1], scale=1.0)
        for u in range(4):
            pz = psum_mm.tile([128, 1024], f32, name="pz", tag="pmm")
            for half in range(2):
                t = 2 * u + half
                for kc in range(16):
                    nc.tensor.matmul(
                        pz[:, half * 512:(half + 1) * 512],
                        hT[kc][:, t * 128:(t + 1) * 128], f2w_t[kc],
                        start=(kc == 0), stop=(kc == 15))
            q1 = work.tile([128, 2 * E], f32, name="q1", tag="wbig")
            nc.vector.tensor_add(q1, pz, MOD["F2B2"])
            nc.vector.tensor_mul(q1, q1, MOD["A22"])
            ot = work.tile([128, 2 * E], f32, name="ot", tag="wbig")
            nc.vector.tensor_add(ot, q1, y2[u])
            for half in range(2):
                t = 2 * u + half
                nc.sync.dma_start(out_d[t * 128:(t + 1) * 128, :],
                                  ot[:, half * E:(half + 1) * E])

    nc.compile()
    return nc


def _get_program():
    if "nc" not in _CACHE:
        _CACHE["nc"] = _build()
    return _CACHE["nc"]


def _pair4(w):
    """[512, C] -> [2, 128, 2, C]: chunk c=2p+j of the contraction dim goes
    to plane j of pair p (DoubleRow layout)."""
    C = w.shape[1]
    return np.ascontiguousarray(
        w.reshape(2, 2, 128, C).transpose(0, 2, 1, 3))


def kernel(**inputs) -> np.ndarray:
    import ml_dtypes
    from concourse.bass_utils import run_bass_kernel_spmd

    fp8 = ml_dtypes.float8_e4m3
    bf16 = ml_dtypes.bfloat16
    ins = {k: np.asarray(v, dtype=np.float32) for k, v in inputs.items()}
    nc = _get_program()

    # host-side fp8 weight prep (shared across cores)
    wq, wk, wv, lvw = ins["wq"], ins["wk"], ins["wv"], ins["lvw"]
    assert np.abs(ins["bq"]).max() == 0 and np.abs(ins["bk"]).max() == 0, (
        "fast path assumes zero attention q/k biases (true for this model)")
    m8 = np.stack([
        _pair4((wq[:, h * E:(h + 1) * E] @ wk[:, h * E:(h + 1) * E].T) * MS)
        for h in range(H)])
    wv8 = np.stack([_pair4(wv[:, h * E:(h + 1) * E] * WS) for h in range(H)])
    lvw8 = np.stack([_pair4(lvw[h * E:(h + 1) * E, :] * WS)
                     for h in range(H)])
    # V bias folded through the (linear) attention average into lvb
    lvb_eff = ins["lvb"].reshape(E) + ins["bv"].reshape(HE) @ lvw

    common = {
        "ln1g": ins["ln1g"].reshape(1, E), "ln1b": ins["ln1b"].reshape(1, E),
        "ln2g": ins["ln2g"].reshape(1, E), "ln2b": ins["ln2b"].reshape(1, E),
        "m8": m8.astype(fp8), "wv8": wv8.astype(fp8),
        "lvw8": lvw8.astype(fp8),
        "f1w16": ins["f1w"].astype(bf16), "f2w16": ins["f2w"].astype(bf16),
        # device multiplies by a1/1024, so pre-scale by 1024
        "lvb": (lvb_eff * (WS * WS)).reshape(1, E).astype(np.float32),
        "f1b": ins["f1b"].reshape(1, FF),
        "f2b": ins["f2b"].reshape(1, E),
        "ident": np.eye(128, dtype=np.float32).astype(bf16),
    }
    for nm in ["g1", "be1", "a1", "g2", "be2", "a2"]:
        common[f"{nm}w"] = ins[f"{nm}w"]
        common[f"{nm}b"] = ins[f"{nm}b"].reshape(1, E)

    in_maps = []
    for b in range(B):
        m = dict(common)
        m["x"] = ins["x"][b]
        m["cond"] = ins["cond"][b].reshape(E, 1)
        in_maps.append(m)

    res = run_bass_kernel_spmd(nc, in_maps, list(range(N_CORES)),
                               trace=TRACE, tmpdir=TRACE_DIR)
    _CACHE["last_result"] = res
    out = np.stack([res.results[b]["out"] for b in range(B)], axis=0)
    return out


# revision 14
# speedup vs baseline: 1.0662x; 1.0662x over previous
"""DiT block kernel for Trainium2, data-parallel over batch across 8 NeuronCores.

Problem: nn_DiTBlock — B=8, S=1024, E=512, H=8 (head_dim = E = 512).
Sharding: batch element b -> core b. Each core runs the full DiT block on its
(S, E) slice with replicated weights; no collectives.

Strategy vs the f32r baseline (618us):
- All attention matmuls run fp8e4 with MatmulPerfMode.DoubleRow (2 contraction
  chunks of 128 packed in the free dims; ~2x the bf16/f32r PE rate). Weights
  are host-cast to fp8 (scaled x32/x64 into e4m3's normal range; the scales
  statically fold into drain ops).
- The K projection is eliminated: host precomputes M_h = wq_h @ wk_h^T (valid
  because bq/bk are structurally zero here), so scores = (y @ M_h) @ y^T with
  y^T as the stationary for every head.
- Softmax denominators come from the PE via an all-ones fp8 stationary
  (replicated across all 128 partitions), not a DVE add tree.
- V bias is folded through the linear attention average into lvb (host).
- alpha1 is applied once at the end (attn_acc accumulates raw o@lvw over
  heads), so lvw needs no per-head device-side scaling.
- FFN stays bf16 (fp8 there pushed max rel err to ~1.9e-2; bf16 FFN lands
  ~9e-3 with fp8 attention). f1 halves are interleaved into the last head's
  LN2 chain so the PE keeps busy across the tail.
- PSUM: [128,512] bank tiles, 6-deep rotation (deeper pipelining beat wider
  1024-drains by ~35us). start=True zeroes a whole bank, so a bank's second
  region rides the first region's pending-zero with start=False.
- Drains: ACT takes exp + V, DVE takes A/oT/acc/recip; GpSimd (no PSUM
  access, slow elementwise) only broadcasts, light SBUF ops, one DMA queue.
"""
import sys
import numpy as np

sys.path.insert(0, '/opt/trn_rl_repo')

B, S, E, H = 8, 1024, 512, 8
HE = H * E          # 4096
FF = 4 * E          # 2048
EPS = 1e-5
SCALE = 1.0 / 32.0  # 1/sqrt(S)
WS = 32.0           # static fp8 scale for wv, lvw
MS = 64.0           # static fp8 scale for M_h = wq_h @ wk_h^T
N_CORES = 8

TRACE = False       # set by test harness to capture an NTFF profile
TRACE_DIR = None

_CACHE = {}


def _build():
    from contextlib import ExitStack
    import concourse.bass as bass
    import concourse.tile as tile
    from concourse import bacc, mybir
    f32 = mybir.dt.float32
    bf16 = mybir.dt.bfloat16
    fp8 = mybir.dt.float8e4
    AF = mybir.ActivationFunctionType
    ALU = mybir.AluOpType
    DR = mybir.MatmulPerfMode.DoubleRow

    nc = bacc.Bacc("TRN2", target_bir_lowering=False, debug=False,
                   num_devices=N_CORES)

    # ---- DRAM parameters --------------------------------------------------
    x_d = nc.dram_tensor("x", [S, E], f32, kind="ExternalInput").ap()
    cond_d = nc.dram_tensor("cond", [E, 1], f32, kind="ExternalInput").ap()

    adaln_w = {}
    adaln_b = {}
    for nm in ["g1", "be1", "a1", "g2", "be2", "a2"]:
        adaln_w[nm] = nc.dram_tensor(f"{nm}w", [E, E], f32,
                                     kind="ExternalInput").ap()
        adaln_b[nm] = nc.dram_tensor(f"{nm}b", [1, E], f32,
                                     kind="ExternalInput").ap()
    ln1g_d = nc.dram_tensor("ln1g", [1, E], f32, kind="ExternalInput").ap()
    ln1b_d = nc.dram_tensor("ln1b", [1, E], f32, kind="ExternalInput").ap()
    ln2g_d = nc.dram_tensor("ln2g", [1, E], f32, kind="ExternalInput").ap()
    ln2b_d = nc.dram_tensor("ln2b", [1, E], f32, kind="ExternalInput").ap()
    # fp8 paired-layout attention weights (host-prepared; see kernel())
    m8_d = nc.dram_tensor("m8", [H, 2, 128, 2, E], fp8,
                          kind="ExternalInput").ap()
    wv8_d = nc.dram_tensor("wv8", [H, 2, 128, 2, E], fp8,
                           kind="ExternalInput").ap()
    lvw8_d = nc.dram_tensor("lvw8", [H, 2, 128, 2, E], fp8,
                            kind="ExternalInput").ap()
    # bf16 FFN weights
    f1w_d = nc.dram_tensor("f1w16", [E, FF], bf16, kind="ExternalInput").ap()
    f2w_d = nc.dram_tensor("f2w16", [FF, E], bf16, kind="ExternalInput").ap()
    lvb_d = nc.dram_tensor("lvb", [1, E], f32, kind="ExternalInput").ap()
    f1b_d = nc.dram_tensor("f1b", [1, FF], f32, kind="ExternalInput").ap()
    f2b_d = nc.dram_tensor("f2b", [1, E], f32, kind="ExternalInput").ap()
    ident_d = nc.dram_tensor("ident", [128, 128], bf16,
                             kind="ExternalInput").ap()
    out_d = nc.dram_tensor("out", [S, E], f32, kind="ExternalOutput").ap()

    with tile.TileContext(nc) as tc, ExitStack() as ctx:
        const = ctx.enter_context(tc.tile_pool(name="const", bufs=1))
        work = ctx.enter_context(tc.tile_pool(name="work", bufs=3))
        psum_mm = ctx.enter_context(
            tc.tile_pool(name="psum_mm", bufs=6, space="PSUM"))
        psum_tp = ctx.enter_context(
            tc.tile_pool(name="psum_tp", bufs=1, space="PSUM"))
        psum_row = ctx.enter_context(
            tc.tile_pool(name="psum_row", bufs=1, space="PSUM"))

        # ---- constants ----
        ident = const.tile([128, 128], bf16)
        nc.sync.dma_start(ident, ident_d)
        eps_t = const.tile([128, 1], f32)
        nc.vector.memset(eps_t, EPS)
        ones8 = const.tile([128, 2, 128], fp8)
        nc.vector.memset(ones8, 1.0)

        cond_cols = const.tile([128, 4], f32)
        nc.sync.dma_start(cond_cols, cond_d.rearrange("(c p) o -> p (c o)", p=128))

        adp = ctx.enter_context(tc.tile_pool(name="adp", bufs=2))

        def adaln_cols(nm):
            """(cond @ W + b) laid out as [128, 4] e-columns (PE matvec)."""
            pcol = psum_row.tile([128, 4], f32, name=f"pcol_{nm}", tag="prow")
            for kc in range(4):
                adw = adp.tile([128, E], f32, name=f"adw_{nm}_{kc}", tag="adw")
                nc.sync.dma_start(adw, adaln_w[nm][kc * 128:(kc + 1) * 128, :])
                for ec in range(4):
                    nc.tensor.matmul(
                        pcol[:, ec:ec + 1],
                        adw[:, ec * 128:(ec + 1) * 128],
                        cond_cols[:, kc:kc + 1],
                        start=(kc == 0 and ec == 0),
                        stop=(kc == 3 and ec == 3))
            bcol = adp.tile([128, 4], f32, name=f"bcol_{nm}", tag="bcol")
            nc.sync.dma_start(
                bcol, adaln_b[nm].rearrange("o (c p) -> (o p) c", p=128))
            mcol = const.tile([128, 4], f32, name=f"mcol_{nm}")
            nc.vector.tensor_add(mcol, pcol, bcol)
            return mcol

        def adaln_rep(nm):
            """(cond @ W + b) replicated to [128, E] (row matvec + bcast)."""
            prow = psum_row.tile([1, E], f32, name=f"prow_{nm}", tag="prow")
            for kc in range(4):
                adw = adp.tile([128, E], f32, name=f"adw_{nm}_{kc}", tag="adw")
                nc.sync.dma_start(adw, adaln_w[nm][kc * 128:(kc + 1) * 128, :])
                nc.tensor.matmul(prow, cond_cols[:, kc:kc + 1], adw,
                                 start=(kc == 0), stop=(kc == 3))
            brow = adp.tile([1, E], f32, name=f"brow_{nm}", tag="brow")
            nc.sync.dma_start(brow, adaln_b[nm])
            arow = adp.tile([1, E], f32, name=f"arow_{nm}", tag="arow")
            nc.vector.tensor_add(arow, prow, brow)
            arep = const.tile([128, E], f32, name=f"arep_{nm}")
            nc.gpsimd.partition_broadcast(arep, arow)
            return arep

        def scale_shift(li, gcol, bcol, lng_d, lnb_d):
            """sc = ln_g*(1+gamma), bi = ln_b*(1+gamma)+beta, as [128,4] cols."""
            gp = const.tile([128, 4], f32, name=f"gp_{li}")
            nc.vector.tensor_scalar_add(gp, gcol, 1.0)
            lgc = adp.tile([128, 4], f32, name=f"lgc_{li}", tag="lgc")
            nc.sync.dma_start(lgc, lng_d.rearrange("o (c p) -> (o p) c", p=128))
            lbc = adp.tile([128, 4], f32, name=f"lbc_{li}", tag="lbc")
            nc.sync.dma_start(lbc, lnb_d.rearrange("o (c p) -> (o p) c", p=128))
            sc = const.tile([128, 4], f32, name=f"sc_{li}")
            nc.vector.tensor_mul(sc, lgc, gp)
            bi = const.tile([128, 4], f32, name=f"bi_{li}")
            nc.vector.tensor_mul(bi, lbc, gp)
            nc.vector.tensor_add(bi, bi, bcol)
            return sc, bi

        # Only g1/be1 gate the LN1->y^T critical path; defer the rest.
        sc1, bi1 = scale_shift(0, adaln_cols("g1"), adaln_cols("be1"),
                               ln1g_d, ln1b_d)

        # ---- persistent activation tiles ----
        # yT8[p][:, j, s] = y^T[e, s] for e-chunk 2p+j (fp8, unscaled)
        yT8 = [const.tile([128, 2, S], fp8, name=f"yT8_{p}") for p in range(2)]
        attn_acc = [const.tile([128, E], f32, name=f"acc{t}") for t in range(8)]
        y2 = [const.tile([128, E], f32, name=f"y2_{t}") for t in range(8)]
        zT16 = [const.tile([128, S], bf16, name=f"zT{c}") for c in range(4)]

        def ln_stats(x_t, tagp):
            st = work.tile([128, 6], f32, name=f"st_{tagp}", tag=f"st_{tagp}")
            nc.vector.bn_stats(st, x_t)
            mv = work.tile([128, 2], f32, name=f"mv_{tagp}", tag=f"mv_{tagp}")
            nc.vector.bn_aggr(mv, st)
            rs = work.tile([128, 1], f32, name=f"rs_{tagp}", tag=f"rs_{tagp}")
            nc.scalar.activation(rs, mv[:, 1:2], AF.Sqrt, bias=eps_t, scale=1.0)
            nc.vector.reciprocal(rs, rs)
            xn = work.tile([128, E], bf16, name=f"xn_{tagp}", tag="wxn")
            nc.vector.tensor_scalar(xn, x_t, scalar1=mv[:, 0:1], scalar2=rs,
                                    op0=ALU.subtract, op1=ALU.mult)
            return xn

        def ln_transpose_pair(xns, scol, bcol, dst_cb, tagp):
            """PE-transpose a pair of normalized tiles; fused modulate ACT."""
            for ec in range(4):
                tp = psum_tp.tile([128, 256], bf16, name=f"tp_{tagp}",
                                  tag="ptp")
                nc.tensor.matmul(tp[:, 0:128],
                                 xns[0][:, ec * 128:(ec + 1) * 128], ident,
                                 is_transpose=True, start=True, stop=False)
                nc.tensor.matmul(tp[:, 128:256],
                                 xns[1][:, ec * 128:(ec + 1) * 128], ident,
                                 is_transpose=True, start=False, stop=True)
                nc.scalar.activation(
                    dst_cb(ec), tp, AF.Identity,
                    bias=bcol[:, ec:ec + 1], scale=scol[:, ec:ec + 1])

        # FFN weights prefetched in chunks during heads 4-7.
        ffp = ctx.enter_context(tc.tile_pool(name="ffp", bufs=1))
        f1w_t = [ffp.tile([128, FF], bf16, name=f"f1w{kc}", tag=f"f1w{kc}")
                 for kc in range(4)]
        f1bc = ffp.tile([128, 16], f32, tag="f1bc")
        f2w_t = [ffp.tile([128, E], bf16, name=f"f2w{kc}", tag=f"f2w{kc}")
                 for kc in range(16)]
        hT = [ffp.tile([128, S], bf16, name=f"hT{hc}", tag=f"hT{hc}")
              for hc in range(16)]

        def emit_ffw_quarter(q):
            for kc in range(4):
                nc.sync.dma_start(
                    f1w_t[kc][:, q * 512:(q + 1) * 512],
                    f1w_d[kc * 128:(kc + 1) * 128, q * 512:(q + 1) * 512])
            for kc in range(4):
                nc.sync.dma_start(
                    f2w_t[4 * q + kc],
                    f2w_d[(4 * q + kc) * 128:(4 * q + kc + 1) * 128, :])
            if q == 3:
                nc.sync.dma_start(
                    f1bc, f1b_d.rearrange("o (c p) -> (o p) c", p=128))

        def emit_f1(sh):
            """f1 + relu for one 512-wide s-half (needs zT16[:, sh*512:...])."""
            for hc in range(16):
                pf = psum_mm.tile([128, 512], f32, name="pf", tag="pmm")
                for kc in range(4):
                    nc.tensor.matmul(
                        pf, f1w_t[kc][:, hc * 128:(hc + 1) * 128],
                        zT16[kc][:, sh * 512:(sh + 1) * 512],
                        start=(kc == 0), stop=(kc == 3))
                nc.scalar.activation(
                    hT[hc][:, sh * 512:(sh + 1) * 512], pf, AF.Relu,
                    bias=f1bc[:, hc:hc + 1], scale=1.0)

        hp_ctx = ExitStack()
        hp = hp_ctx.enter_context(tc.tile_pool(name="hp", bufs=1))

        def emit_head_dma(h):
            """Load M, wv, lvw fp8 pair tiles for head h."""
            m_t = [hp.tile([128, 2, E], fp8, name=f"m{p}", tag=f"m{p}_{h % 2}")
                   for p in range(2)]
            wv_t = [hp.tile([128, 2, E], fp8, name=f"wv{p}",
                            tag=f"wv{p}_{h % 2}") for p in range(2)]
            lvw_t = [hp.tile([128, 2, E], fp8, name=f"lvw{p}",
                             tag=f"lvw{p}_{h % 2}") for p in range(2)]
            for p in range(2):
                nc.gpsimd.dma_start(m_t[p], m8_d[h, p])
                nc.sync.dma_start(wv_t[p], wv8_d[h, p])
                nc.sync.dma_start(lvw_t[p], lvw8_d[h, p])
            return m_t, wv_t, lvw_t

        MOD = {}

        def emit_adaln_a1_seed():
            A1 = adaln_rep("a1")
            # attention branch applied at the end: y2 = (x + lvb_eff*a1)
            #                                          + attn_acc * (a1/1024)
            A1c = const.tile([128, E], f32)
            nc.vector.tensor_scalar(A1c, A1, scalar1=1.0 / (WS * WS),
                                    scalar2=None, op0=ALU.mult)
            MOD["A1c"] = A1c
            LVBA = const.tile([128, E], f32)
            nc.sync.dma_start(LVBA, lvb_d.broadcast_to([128, E]))
            nc.gpsimd.tensor_tensor(LVBA, LVBA, A1, op=ALU.mult)
            for t in range(8):
                x_t3 = work.tile([128, E], f32, name="xt3", tag="wbig")
                nc.sync.dma_start(x_t3, x_d[t * 128:(t + 1) * 128, :])
                nc.gpsimd.tensor_tensor(y2[t], x_t3, LVBA, op=ALU.add)

        def emit_adaln_ln2():
            MOD["sc2"], MOD["bi2"] = scale_shift(
                1, adaln_cols("g2"), adaln_cols("be2"), ln2g_d, ln2b_d)
            MOD["A2"] = adaln_rep("a2")
            F2B = const.tile([128, E], f32)
            nc.sync.dma_start(F2B, f2b_d.broadcast_to([128, E]))
            MOD["F2B"] = F2B

        # LN2 emitted per tile from inside the last head's lv loop, so its
        # DVE chain hides under remaining matmuls; f1 halves are interleaved
        # as soon as their zT16 columns are complete.
        _ln2_pend = []

        def ln2_tile(t):
            # finalize y2[t] = (x + lvb_eff*a1) + attn_acc*(a1/1024)
            t1 = work.tile([128, E], f32, name="t1", tag="wbig")
            nc.vector.tensor_mul(t1, attn_acc[t], MOD["A1c"])
            nc.vector.tensor_add(y2[t], y2[t], t1)
            _ln2_pend.append((t, ln_stats(y2[t], "ln2")))
            if len(_ln2_pend) < 2:
                return
            (ta, xa), (tb, xb) = _ln2_pend
            _ln2_pend.clear()
            ln_transpose_pair(
                [xa, xb], MOD["sc2"], MOD["bi2"],
                lambda ec: zT16[ec][:, ta * 128:(ta + 2) * 128], "ln2")

        # ---- Phase 1: LN1 -> y^T (fp8 paired planes) ----
        PRE0 = emit_head_dma(0)
        for t0 in range(0, 8, 2):
            xns = []
            for t in (t0, t0 + 1):
                x_t = work.tile([128, E], f32, name="xt_ln1", tag="wbig")
                nc.sync.dma_start(x_t, x_d[t * 128:(t + 1) * 128, :])
                xns.append(ln_stats(x_t, "ln1"))
            ln_transpose_pair(
                xns, sc1, bi1,
                lambda ec: yT8[ec // 2][:, ec % 2, t0 * 128:(t0 + 2) * 128],
                "ln1")

        # ---- Phase 2: attention heads (software-pipelined) ----
        # Two 256-wide DoubleRow regions share each [128,512] psum bank; only
        # the bank's first matmul sets start (start zeroes the whole bank).
        def dr_accum(ps2, stat_fn, mov_fn, npass):
            for k in range(2):
                for reg in range(2):
                    for q in range(npass):
                        nc.tensor.matmul(
                            ps2[k][:, reg * 256:(reg + 1) * 256],
                            stat_fn(q), mov_fn(q, 2 * k + reg),
                            start=(q == 0 and reg == 0), stop=(q == npass - 1),
                            perf_mode=DR)

        def head_A(h, m_t):
            """A^T = (y @ M_h*64)^T, fp8 paired planes. Drain: DVE copy."""
            AT8 = [hp.tile([128, 2, S], fp8, name=f"AT{p}",
                           tag=f"AT{p}_{h % 2}") for p in range(2)]
            for mc in range(4):
                ps2 = [psum_mm.tile([128, 512], f32, name="pa", tag="pmm")
                       for _ in range(2)]
                dr_accum(
                    ps2,
                    lambda p: m_t[p][:, :, mc * 128:(mc + 1) * 128],
                    lambda p, n4: yT8[p][:, :, n4 * 256:(n4 + 1) * 256], 2)
                for k in range(2):
                    nc.vector.tensor_copy(
                        AT8[mc // 2][:, mc % 2, k * 512:(k + 1) * 512], ps2[k])
            return AT8

        def head_v(h, wv_t):
            """V_h*32 (natural layout), fp8 paired planes. Drain: ACT."""
            Vh8 = [hp.tile([128, 2, E], fp8, name=f"V{tp}",
                           tag=f"V{tp}_{h % 2}") for tp in range(4)]
            for t in range(8):
                pv = psum_mm.tile([128, E], f32, name="pv", tag="pmm")
                for c in range(2):
                    for p in range(2):
                        nc.tensor.matmul(
                            pv[:, c * 256:(c + 1) * 256],
                            yT8[p][:, :, t * 128:(t + 1) * 128],
                            wv_t[p][:, :, c * 256:(c + 1) * 256],
                            start=(p == 0 and c == 0), stop=(p == 1),
                            perf_mode=DR)
                nc.scalar.activation(Vh8[t // 2][:, t % 2, :], pv,
                                     AF.Identity, scale=1.0)
            return Vh8

        def head_scores(h, AT8):
            """scores^T = y A^T (contract head dim); exp via ACT -> fp8."""
            Eh8 = [hp.tile([128, 2, S], fp8, name=f"E{tp}",
                           tag=f"E{tp}_{h % 2}") for tp in range(4)]
            for t in range(8):
                ps2 = [psum_mm.tile([128, 512], f32, name="ps", tag="pmm")
                       for _ in range(2)]
                dr_accum(
                    ps2,
                    lambda p: yT8[p][:, :, t * 128:(t + 1) * 128],
                    lambda p, n4: AT8[p][:, :, n4 * 256:(n4 + 1) * 256], 2)
                for k in range(2):
                    nc.scalar.activation(
                        Eh8[t // 2][:, t % 2, k * 512:(k + 1) * 512], ps2[k],
                        AF.Exp, scale=SCALE / MS)
            return Eh8

        def head_sums(h, Eh8):
            """Softmax denominators replicated across partitions (all-ones
            fp8 stationary); reciprocal on DVE."""
            Rrep = hp.tile([128, S], f32, tag=f"rrep_{h % 2}")
            ps2 = [psum_mm.tile([128, 512], f32, name="pr", tag="pmm")
                   for _ in range(2)]
            dr_accum(
                ps2,
                lambda tp: ones8,
                lambda tp, n4: Eh8[tp][:, :, n4 * 256:(n4 + 1) * 256], 4)
            for k in range(2):
                nc.vector.reciprocal_approx_fast(
                    Rrep[:, k * 512:(k + 1) * 512], ps2[k])
            return Rrep

        def head_av(h, Vh8, Eh8, Rrep):
            """o^T*32 = (V*32)^T E^T / sums, fp8. Drain: DVE TT mult."""
            oT8 = [hp.tile([128, 2, S], fp8, name=f"oT{p}",
                           tag=f"oT{p}_{h % 2}") for p in range(2)]
            for ec in range(4):
                ps2 = [psum_mm.tile([128, 512], f32, name="po", tag="pmm")
                       for _ in range(2)]
                dr_accum(
                    ps2,
                    lambda tp: Vh8[tp][:, :, ec * 128:(ec + 1) * 128],
                    lambda tp, n4: Eh8[tp][:, :, n4 * 256:(n4 + 1) * 256], 4)
                for k in range(2):
                    nc.vector.tensor_tensor(
                        oT8[ec // 2][:, ec % 2, k * 512:(k + 1) * 512],
                        ps2[k], Rrep[:, k * 512:(k + 1) * 512], op=ALU.mult)
            return oT8

        def head_lv(h, oT8, lvw_t, last):
            """attn_acc += (o^T*32)^T @ (lvw*32); scale folded at the end."""
            for t in range(8):
                pl = psum_mm.tile([128, E], f32, name="pl", tag="pmm")
                for c in range(2):
                    for p in range(2):
                        nc.tensor.matmul(
                            pl[:, c * 256:(c + 1) * 256],
                            oT8[p][:, :, t * 128:(t + 1) * 128],
                            lvw_t[p][:, :, c * 256:(c + 1) * 256],
                            start=(p == 0 and c == 0), stop=(p == 1),
                            perf_mode=DR)
                if h == 0:
                    nc.vector.tensor_copy(attn_acc[t], pl)
                else:
                    nc.vector.tensor_add(attn_acc[t], attn_acc[t], pl)
                if last:
                    ln2_tile(t)
                    if t == 3:
                        emit_f1(0)
                    elif t == 7:
                        emit_f1(1)

        # Pipelined loop. PE stream per iteration:
        #   A(h) -> V(h) -> scores(h) -> sums(h-1) -> AV(h-1) -> lv(h-1)
        prev = None
        for h in range(H):
            m_t, wv_t, lvw_t = PRE0 if h == 0 else emit_head_dma(h)
            AT8 = head_A(h, m_t)
            Vh8 = head_v(h, wv_t)
            Eh8 = head_scores(h, AT8)
            if prev is not None:
                ph, pV, pE, plvw = prev
                Rrep = head_sums(ph, pE)
                oT8 = head_av(ph, pV, pE, Rrep)
                head_lv(ph, oT8, plvw, last=False)
            if h == 0:
                emit_adaln_a1_seed()
            elif h == 2:
                emit_adaln_ln2()
            elif h >= 4:
                emit_ffw_quarter(h - 4)
            prev = (h, Vh8, Eh8, lvw_t)
        ph, pV, pE, plvw = prev
        Rrep = head_sums(ph, pE)
        oT8 = head_av(ph, pV, pE, Rrep)
        head_lv(ph, oT8, plvw, last=True)
        hp_ctx.close()

        # ---- Phase 5: FFN second matmul (f1 already emitted above) ----
        for t in range(8):
            pz = psum_mm.tile([128, E], f32, name="pz", tag="pmm")
            for kc in range(16):
                nc.tensor.matmul(
                    pz, hT[kc][:, t * 128:(t + 1) * 128], f2w_t[kc],
                    start=(kc == 0), stop=(kc == 15))
            q1 = work.tile([128, E], f32, name="q1", tag="wbig")
            nc.vector.tensor_add(q1, pz, MOD["F2B"])
            nc.vector.tensor_mul(q1, q1, MOD["A2"])
            ot = work.tile([128, E], f32, name="ot", tag="wbig")
            nc.vector.tensor_add(ot, q1, y2[t])
            nc.sync.dma_start(out_d[t * 128:(t + 1) * 128, :], ot)

    nc.compile()
    return nc


def _get_program():
    if "nc" not in _CACHE:
        _CACHE["nc"] = _build()
    return _CACHE["nc"]


def _pair4(w):
    """[512, C] -> [2, 128, 2, C]: chunk c=2p+j of the contraction dim goes
    to plane j of pair p (DoubleRow layout)."""
    C = w.shape[1]
    return np.ascontiguousarray(
        w.reshape(2, 2, 128, C).transpose(0, 2, 1, 3))


def kernel(**inputs) -> np.ndarray:
    import ml_dtypes
    from concourse.bass_utils import run_bass_kernel_spmd

    fp8 = ml_dtypes.float8_e4m3
    bf16 = ml_dtypes.bfloat16
    ins = {k: np.asarray(v, dtype=np.float32) for k, v in inputs.items()}
    nc = _get_program()

    # host-side fp8 weight prep (shared across cores)
    wq, wk, wv, lvw = ins["wq"], ins["wk"], ins["wv"], ins["lvw"]
    assert np.abs(ins["bq"]).max() == 0 and np.abs(ins["bk"]).max() == 0, (
        "fast path assumes zero attention q/k biases (true for this model)")
    m8 = np.stack([
        _pair4((wq[:, h * E:(h + 1) * E] @ wk[:, h * E:(h + 1) * E].T) * MS)
        for h in range(H)])
    wv8 = np.stack([_pair4(wv[:, h * E:(h + 1) * E] * WS) for h in range(H)])
    lvw8 = np.stack([_pair4(lvw[h * E:(h + 1) * E, :] * WS)
                     for h in range(H)])
    # V bias folded through the (linear) attention average into lvb
    lvb_eff = ins["lvb"].reshape(E) + ins["bv"].reshape(HE) @ lvw

    common = {
        "ln1g": ins["ln1g"].reshape(1, E), "ln1b": ins["ln1b"].reshape(1, E),
        "ln2g": ins["ln2g"].reshape(1, E), "ln2b": ins["ln2b"].reshape(1, E),
        "m8": m8.astype(fp8), "wv8": wv8.astype(fp8),
        "lvw8": lvw8.astype(fp8),
        "f1w16": ins["f1w"].astype(bf16), "f2w16": ins["f2w"].astype(bf16),
        "lvb": lvb_eff.reshape(1, E).astype(np.float32),
        "f1b": ins["f1b"].reshape(1, FF),
        "f2b": ins["f2b"].reshape(1, E),
        "ident": np.eye(128, dtype=np.float32).astype(bf16),
    }
    for nm in ["g1", "be1", "a1", "g2", "be2", "a2"]:
        common[f"{nm}w"] = ins[f"{nm}w"]
        common[f"{nm}b"] = ins[f"{nm}b"].reshape(1, E)

    in_maps = []
    for b in range(B):
        m = dict(common)
        m["x"] = ins["x"][b]
        m["cond"] = ins["cond"][b].reshape(E, 1)
        in_maps.append(m)

    res = run_bass_kernel_spmd(nc, in_maps, list(range(N_CORES)),
                               trace=TRACE, tmpdir=TRACE_DIR)
    _CACHE["last_result"] = res
    out = np.stack([res.results[b]["out"] for b in range(B)], axis=0)
    return out


# revision 18
# speedup vs baseline: 1.1500x; 1.0786x over previous
"""DiT block kernel for Trainium2, data-parallel over batch across 8 NeuronCores.

Problem: nn_DiTBlock — B=8, S=1024, E=512, H=8 (head_dim = E = 512).
Sharding: batch element b -> core b. Each core runs the full DiT block on its
(S, E) slice with replicated weights; no collectives.

Strategy vs the f32r baseline (618us):
- All attention matmuls run fp8e4 with MatmulPerfMode.DoubleRow (2 contraction
  chunks of 128 packed in the free dims; ~2x the bf16/f32r PE rate). Weights
  are host-cast to fp8 (scaled x32/x64 into e4m3's normal range; the scales
  statically fold into drain ops).
- The K projection is eliminated: host precomputes M_h = wq_h @ wk_h^T (valid
  because bq/bk are structurally zero here), so scores = (y @ M_h) @ y^T with
  y^T as the stationary for every head.
- Softmax denominators come from the PE via an all-ones fp8 stationary
  (replicated across all 128 partitions), not a DVE add tree.
- V bias is folded through the linear attention average into lvb (host).
- alpha1 is applied once at the end (attn_acc accumulates raw o@lvw over
  heads), so lvw needs no per-head device-side scaling.
- FFN stays bf16 (fp8 there pushed max rel err to ~1.9e-2; bf16 FFN lands
  ~9e-3 with fp8 attention). f1 halves are interleaved into the last head's
  LN2 chain so the PE keeps busy across the tail.
- PSUM: [128,512] bank tiles, 6-deep rotation (deeper pipelining beat wider
  1024-drains by ~35us). start=True zeroes a whole bank, so a bank's second
  region rides the first region's pending-zero with start=False.
- Drains: ACT takes exp + V, DVE takes A/oT/acc/recip; GpSimd (no PSUM
  access, slow elementwise) only broadcasts, light SBUF ops, one DMA queue.
"""
import sys
import numpy as np

sys.path.insert(0, '/opt/trn_rl_repo')

B, S, E, H = 8, 1024, 512, 8
HE = H * E          # 4096
FF = 4 * E          # 2048
EPS = 1e-5
SCALE = 1.0 / 32.0  # 1/sqrt(S)
WS = 32.0           # static fp8 scale for wv, lvw
MS = 64.0           # static fp8 scale for M_h = wq_h @ wk_h^T
N_CORES = 8

TRACE = False       # set by test harness to capture an NTFF profile
TRACE_DIR = None

_CACHE = {}


def _build():
    from contextlib import ExitStack
    import concourse.bass as bass
    import concourse.tile as tile
    from concourse import bacc, mybir
    f32 = mybir.dt.float32
    bf16 = mybir.dt.bfloat16
    fp8 = mybir.dt.float8e4
    AF = mybir.ActivationFunctionType
    ALU = mybir.AluOpType
    DR = mybir.MatmulPerfMode.DoubleRow

    nc = bacc.Bacc("TRN2", target_bir_lowering=False, debug=False,
                   num_devices=N_CORES)

    # ---- DRAM parameters --------------------------------------------------
    x_d = nc.dram_tensor("x", [S, E], f32, kind="ExternalInput").ap()
    cond_d = nc.dram_tensor("cond", [E, 1], f32, kind="ExternalInput").ap()

    adaln_w = {}
    adaln_b = {}
    for nm in ["g1", "be1", "a1", "g2", "be2", "a2"]:
        adaln_w[nm] = nc.dram_tensor(f"{nm}w", [E, E], bf16,
                                     kind="ExternalInput").ap()
        adaln_b[nm] = nc.dram_tensor(f"{nm}b", [1, E], f32,
                                     kind="ExternalInput").ap()
    ln1g_d = nc.dram_tensor("ln1g", [1, E], f32, kind="ExternalInput").ap()
    ln1b_d = nc.dram_tensor("ln1b", [1, E], f32, kind="ExternalInput").ap()
    ln2g_d = nc.dram_tensor("ln2g", [1, E], f32, kind="ExternalInput").ap()
    ln2b_d = nc.dram_tensor("ln2b", [1, E], f32, kind="ExternalInput").ap()
    # fp8 paired-layout attention weights (host-prepared; see kernel())
    m8_d = nc.dram_tensor("m8", [H, 2, 128, 2, E], fp8,
                          kind="ExternalInput").ap()
    wv8_d = nc.dram_tensor("wv8", [H, 2, 128, 2, E], fp8,
                           kind="ExternalInput").ap()
    lvw8_d = nc.dram_tensor("lvw8", [H, 2, 128, 2, E], fp8,
                            kind="ExternalInput").ap()
    # bf16 FFN weights
    f1w_d = nc.dram_tensor("f1w16", [E, FF], bf16, kind="ExternalInput").ap()
    f2w_d = nc.dram_tensor("f2w16", [FF, E], bf16, kind="ExternalInput").ap()
    lvb_d = nc.dram_tensor("lvb", [1, E], f32, kind="ExternalInput").ap()
    f1b_d = nc.dram_tensor("f1b", [1, FF], f32, kind="ExternalInput").ap()
    f2b_d = nc.dram_tensor("f2b", [1, E], f32, kind="ExternalInput").ap()
    ident_d = nc.dram_tensor("ident", [128, 128], bf16,
                             kind="ExternalInput").ap()
    out_d = nc.dram_tensor("out", [S, E], f32, kind="ExternalOutput").ap()

    with tile.TileContext(nc) as tc, ExitStack() as ctx:
        const = ctx.enter_context(tc.tile_pool(name="const", bufs=1))
        work = ctx.enter_context(tc.tile_pool(name="work", bufs=3))
        psum_mm = ctx.enter_context(
            tc.tile_pool(name="psum_mm", bufs=6, space="PSUM"))
        psum_tp = ctx.enter_context(
            tc.tile_pool(name="psum_tp", bufs=1, space="PSUM"))
        psum_row = ctx.enter_context(
            tc.tile_pool(name="psum_row", bufs=1, space="PSUM"))

        # ---- constants ----
        ident = const.tile([128, 128], bf16)
        nc.sync.dma_start(ident, ident_d)
        eps_t = const.tile([128, 1], f32)
        nc.vector.memset(eps_t, EPS)
        ones8 = const.tile([128, 2, 128], fp8)
        nc.vector.memset(ones8, 1.0)

        cond_f = const.tile([128, 4], f32)
        nc.sync.dma_start(cond_f, cond_d.rearrange("(c p) o -> p (c o)", p=128))
        cond_cols = const.tile([128, 4], bf16)
        nc.vector.tensor_copy(cond_cols, cond_f)

        adp = ctx.enter_context(tc.tile_pool(name="adp", bufs=2))

        def adaln_cols(nm):
            """(cond @ W + b) laid out as [128, 4] e-columns (PE matvec)."""
            pcol = psum_row.tile([128, 4], f32, name=f"pcol_{nm}", tag="prow")
            for kc in range(4):
                adw = adp.tile([128, E], bf16, name=f"adw_{nm}_{kc}",
                               tag="adw")
                nc.sync.dma_start(adw, adaln_w[nm][kc * 128:(kc + 1) * 128, :])
                for ec in range(4):
                    nc.tensor.matmul(
                        pcol[:, ec:ec + 1],
                        adw[:, ec * 128:(ec + 1) * 128],
                        cond_cols[:, kc:kc + 1],
                        start=(kc == 0 and ec == 0),
                        stop=(kc == 3 and ec == 3))
            bcol = adp.tile([128, 4], f32, name=f"bcol_{nm}", tag="bcol")
            nc.sync.dma_start(
                bcol, adaln_b[nm].rearrange("o (c p) -> (o p) c", p=128))
            mcol = const.tile([128, 4], f32, name=f"mcol_{nm}")
            nc.vector.tensor_add(mcol, pcol, bcol)
            return mcol

        def adaln_rep(nm):
            """(cond @ W + b) replicated to [128, E] (row matvec + bcast)."""
            prow = psum_row.tile([1, E], f32, name=f"prow_{nm}", tag="prow")
            for kc in range(4):
                adw = adp.tile([128, E], bf16, name=f"adw_{nm}_{kc}",
                               tag="adw")
                nc.sync.dma_start(adw, adaln_w[nm][kc * 128:(kc + 1) * 128, :])
                nc.tensor.matmul(prow, cond_cols[:, kc:kc + 1], adw,
                                 start=(kc == 0), stop=(kc == 3))
            brow = adp.tile([1, E], f32, name=f"brow_{nm}", tag="brow")
            nc.sync.dma_start(brow, adaln_b[nm])
            arow = adp.tile([1, E], f32, name=f"arow_{nm}", tag="arow")
            nc.vector.tensor_add(arow, prow, brow)
            arep = const.tile([128, E], f32, name=f"arep_{nm}")
            nc.gpsimd.partition_broadcast(arep, arow)
            return arep

        def scale_shift(li, gcol, bcol, lng_d, lnb_d):
            """sc = ln_g*(1+gamma), bi = ln_b*(1+gamma)+beta, as [128,4] cols."""
            gp = const.tile([128, 4], f32, name=f"gp_{li}")
            nc.vector.tensor_scalar_add(gp, gcol, 1.0)
            lgc = adp.tile([128, 4], f32, name=f"lgc_{li}", tag="lgc")
            nc.sync.dma_start(lgc, lng_d.rearrange("o (c p) -> (o p) c", p=128))
            lbc = adp.tile([128, 4], f32, name=f"lbc_{li}", tag="lbc")
            nc.sync.dma_start(lbc, lnb_d.rearrange("o (c p) -> (o p) c", p=128))
            sc = const.tile([128, 4], f32, name=f"sc_{li}")
            nc.vector.tensor_mul(sc, lgc, gp)
            bi = const.tile([128, 4], f32, name=f"bi_{li}")
            nc.vector.tensor_mul(bi, lbc, gp)
            nc.vector.tensor_add(bi, bi, bcol)
            return sc, bi

        # Only g1/be1 gate the LN1->y^T critical path; defer the rest.
        sc1, bi1 = scale_shift(0, adaln_cols("g1"), adaln_cols("be1"),
                               ln1g_d, ln1b_d)

        # ---- persistent activation tiles ----
        # yT8[p][:, j, s] = y^T[e, s] for e-chunk 2p+j (fp8, unscaled)
        yT8 = [const.tile([128, 2, S], fp8, name=f"yT8_{p}") for p in range(2)]
        attn_acc = [const.tile([128, E], f32, name=f"acc{t}") for t in range(8)]
        y2 = [const.tile([128, E], f32, name=f"y2_{t}") for t in range(8)]
        zT16 = [const.tile([128, S], bf16, name=f"zT{c}") for c in range(4)]

        def ln_stats(x_t, tagp):
            st = work.tile([128, 6], f32, name=f"st_{tagp}", tag=f"st_{tagp}")
            nc.vector.bn_stats(st, x_t)
            mv = work.tile([128, 2], f32, name=f"mv_{tagp}", tag=f"mv_{tagp}")
            nc.vector.bn_aggr(mv, st)
            rs = work.tile([128, 1], f32, name=f"rs_{tagp}", tag=f"rs_{tagp}")
            nc.scalar.activation(rs, mv[:, 1:2], AF.Sqrt, bias=eps_t, scale=1.0)
            nc.vector.reciprocal(rs, rs)
            xn = work.tile([128, E], bf16, name=f"xn_{tagp}", tag="wxn")
            nc.vector.tensor_scalar(xn, x_t, scalar1=mv[:, 0:1], scalar2=rs,
                                    op0=ALU.subtract, op1=ALU.mult)
            return xn

        def ln_transpose_pair(xns, scol, bcol, dst_cb, tagp):
            """PE-transpose a pair of normalized tiles; fused modulate ACT."""
            for ec in range(4):
                tp = psum_tp.tile([128, 256], bf16, name=f"tp_{tagp}",
                                  tag="ptp")
                nc.tensor.matmul(tp[:, 0:128],
                                 xns[0][:, ec * 128:(ec + 1) * 128], ident,
                                 is_transpose=True, start=True, stop=False)
                nc.tensor.matmul(tp[:, 128:256],
                                 xns[1][:, ec * 128:(ec + 1) * 128], ident,
                                 is_transpose=True, start=False, stop=True)
                nc.scalar.activation(
                    dst_cb(ec), tp, AF.Identity,
                    bias=bcol[:, ec:ec + 1], scale=scol[:, ec:ec + 1])

        # FFN weights prefetched in chunks during heads 4-7.
        ffp = ctx.enter_context(tc.tile_pool(name="ffp", bufs=1))
        f1w_t = [ffp.tile([128, FF], bf16, name=f"f1w{kc}", tag=f"f1w{kc}")
                 for kc in range(4)]
        f1bc = ffp.tile([128, 16], f32, tag="f1bc")
        f2w_t = [ffp.tile([128, E], bf16, name=f"f2w{kc}", tag=f"f2w{kc}")
                 for kc in range(16)]
        hT = [ffp.tile([128, S], bf16, name=f"hT{hc}", tag=f"hT{hc}")
              for hc in range(16)]

        def emit_ffw_quarter(q):
            for kc in range(4):
                nc.sync.dma_start(
                    f1w_t[kc][:, q * 512:(q + 1) * 512],
                    f1w_d[kc * 128:(kc + 1) * 128, q * 512:(q + 1) * 512])
            for kc in range(4):
                nc.sync.dma_start(
                    f2w_t[4 * q + kc],
                    f2w_d[(4 * q + kc) * 128:(4 * q + kc + 1) * 128, :])
            if q == 3:
                nc.sync.dma_start(
                    f1bc, f1b_d.rearrange("o (c p) -> (o p) c", p=128))

        def emit_f1(sh):
            """f1 + relu for one 512-wide s-half (needs zT16[:, sh*512:...])."""
            for hc in range(16):
                pf = psum_mm.tile([128, 512], f32, name="pf", tag="pmm")
                for kc in range(4):
                    nc.tensor.matmul(
                        pf, f1w_t[kc][:, hc * 128:(hc + 1) * 128],
                        zT16[kc][:, sh * 512:(sh + 1) * 512],
                        start=(kc == 0), stop=(kc == 3))
                nc.scalar.activation(
                    hT[hc][:, sh * 512:(sh + 1) * 512], pf, AF.Relu,
                    bias=f1bc[:, hc:hc + 1], scale=1.0)

        hp_ctx = ExitStack()
        hp = hp_ctx.enter_context(tc.tile_pool(name="hp", bufs=1))

        def emit_head_dma(h):
            """Load M, wv, lvw fp8 pair tiles for head h."""
            m_t = [hp.tile([128, 2, E], fp8, name=f"m{p}", tag=f"m{p}_{h % 2}")
                   for p in range(2)]
            wv_t = [hp.tile([128, 2, E], fp8, name=f"wv{p}",
                            tag=f"wv{p}_{h % 2}") for p in range(2)]
            lvw_t = [hp.tile([128, 2, E], fp8, name=f"lvw{p}",
                             tag=f"lvw{p}_{h % 2}") for p in range(2)]
            for p in range(2):
                nc.gpsimd.dma_start(m_t[p], m8_d[h, p])
                nc.sync.dma_start(wv_t[p], wv8_d[h, p])
                nc.sync.dma_start(lvw_t[p], lvw8_d[h, p])
            return m_t, wv_t, lvw_t

        MOD = {}

        def emit_adaln_a1_seed():
            A1 = adaln_rep("a1")
            # attention branch applied at the end: y2 = (x + lvb_eff*a1)
            #                                          + attn_acc * (a1/1024)
            A1c = const.tile([128, E], f32)
            nc.vector.tensor_scalar(A1c, A1, scalar1=1.0 / (WS * WS),
                                    scalar2=None, op0=ALU.mult)
            MOD["A1c"] = A1c
            LVBA = const.tile([128, E], f32)
            nc.sync.dma_start(LVBA, lvb_d.broadcast_to([128, E]))
            nc.gpsimd.tensor_tensor(LVBA, LVBA, A1, op=ALU.mult)
            for t in range(8):
                x_t3 = work.tile([128, E], f32, name="xt3", tag="wbig")
                nc.sync.dma_start(x_t3, x_d[t * 128:(t + 1) * 128, :])
                nc.gpsimd.tensor_tensor(y2[t], x_t3, LVBA, op=ALU.add)

        def emit_adaln_ln2():
            MOD["sc2"], MOD["bi2"] = scale_shift(
                1, adaln_cols("g2"), adaln_cols("be2"), ln2g_d, ln2b_d)
            MOD["A2"] = adaln_rep("a2")
            F2B = const.tile([128, E], f32)
            nc.sync.dma_start(F2B, f2b_d.broadcast_to([128, E]))
            MOD["F2B"] = F2B

        # LN2 emitted per tile from inside the last head's lv loop, so its
        # DVE chain hides under remaining matmuls; f1 halves are interleaved
        # as soon as their zT16 columns are complete.
        _ln2_pend = []

        def ln2_tile(t):
            # finalize y2[t] = (x + lvb_eff*a1) + attn_acc*(a1/1024)
            t1 = work.tile([128, E], f32, name="t1", tag="wbig")
            nc.vector.tensor_mul(t1, attn_acc[t], MOD["A1c"])
            nc.vector.tensor_add(y2[t], y2[t], t1)
            _ln2_pend.append((t, ln_stats(y2[t], "ln2")))
            if len(_ln2_pend) < 2:
                return
            (ta, xa), (tb, xb) = _ln2_pend
            _ln2_pend.clear()
            ln_transpose_pair(
                [xa, xb], MOD["sc2"], MOD["bi2"],
                lambda ec: zT16[ec][:, ta * 128:(ta + 2) * 128], "ln2")

        # ---- Phase 1: LN1 -> y^T (fp8 paired planes) ----
        PRE0 = emit_head_dma(0)
        for t0 in range(0, 8, 2):
            xns = []
            for t in (t0, t0 + 1):
                x_t = work.tile([128, E], f32, name="xt_ln1", tag="wbig")
                nc.sync.dma_start(x_t, x_d[t * 128:(t + 1) * 128, :])
                xns.append(ln_stats(x_t, "ln1"))
            ln_transpose_pair(
                xns, sc1, bi1,
                lambda ec: yT8[ec // 2][:, ec % 2, t0 * 128:(t0 + 2) * 128],
                "ln1")

        # ---- Phase 2: attention heads (software-pipelined) ----
        # Two 256-wide DoubleRow regions share each [128,512] psum bank; only
        # the bank's first matmul sets start (start zeroes the whole bank).
        def dr_accum(ps2, stat_fn, mov_fn, npass):
            for k in range(2):
                for reg in range(2):
                    for q in range(npass):
                        nc.tensor.matmul(
                            ps2[k][:, reg * 256:(reg + 1) * 256],
                            stat_fn(q), mov_fn(q, 2 * k + reg),
                            start=(q == 0 and reg == 0), stop=(q == npass - 1),
                            perf_mode=DR)

        def head_A(h, m_t):
            """A^T = (y @ M_h*64)^T, fp8 paired planes. Drain: DVE copy."""
            AT8 = [hp.tile([128, 2, S], fp8, name=f"AT{p}",
                           tag=f"AT{p}_{h % 2}") for p in range(2)]
            for mc in range(4):
                ps2 = [psum_mm.tile([128, 512], f32, name="pa", tag="pmm")
                       for _ in range(2)]
                dr_accum(
                    ps2,
                    lambda p: m_t[p][:, :, mc * 128:(mc + 1) * 128],
                    lambda p, n4: yT8[p][:, :, n4 * 256:(n4 + 1) * 256], 2)
                for k in range(2):
                    nc.vector.tensor_copy(
                        AT8[mc // 2][:, mc % 2, k * 512:(k + 1) * 512], ps2[k])
            return AT8

        def head_v(h, wv_t):
            """V_h*32 (natural layout), fp8 paired planes. Drain: ACT."""
            Vh8 = [hp.tile([128, 2, E], fp8, name=f"V{tp}",
                           tag=f"V{tp}_{h % 2}") for tp in range(4)]
            for t in range(8):
                pv = psum_mm.tile([128, E], f32, name="pv", tag="pmm")
                for c in range(2):
                    for p in range(2):
                        nc.tensor.matmul(
                            pv[:, c * 256:(c + 1) * 256],
                            yT8[p][:, :, t * 128:(t + 1) * 128],
                            wv_t[p][:, :, c * 256:(c + 1) * 256],
                            start=(p == 0 and c == 0), stop=(p == 1),
                            perf_mode=DR)
                nc.scalar.activation(Vh8[t // 2][:, t % 2, :], pv,
                                     AF.Identity, scale=1.0)
            return Vh8

        def head_scores(h, AT8):
            """scores^T = y A^T (contract head dim); exp via ACT -> fp8."""
            Eh8 = [hp.tile([128, 2, S], fp8, name=f"E{tp}",
                           tag=f"E{tp}_{h % 2}") for tp in range(4)]
            for t in range(8):
                ps2 = [psum_mm.tile([128, 512], f32, name="ps", tag="pmm")
                       for _ in range(2)]
                dr_accum(
                    ps2,
                    lambda p: yT8[p][:, :, t * 128:(t + 1) * 128],
                    lambda p, n4: AT8[p][:, :, n4 * 256:(n4 + 1) * 256], 2)
                for k in range(2):
                    nc.scalar.activation(
                        Eh8[t // 2][:, t % 2, k * 512:(k + 1) * 512], ps2[k],
                        AF.Exp, scale=SCALE / MS)
            return Eh8

        def head_sums(h, Eh8):
            """Softmax denominators replicated across partitions (all-ones
            fp8 stationary); reciprocal on DVE."""
            Rrep = hp.tile([128, S], f32, tag=f"rrep_{h % 2}")
            ps2 = [psum_mm.tile([128, 512], f32, name="pr", tag="pmm")
                   for _ in range(2)]
            dr_accum(
                ps2,
                lambda tp: ones8,
                lambda tp, n4: Eh8[tp][:, :, n4 * 256:(n4 + 1) * 256], 4)
            for k in range(2):
                nc.vector.reciprocal_approx_fast(
                    Rrep[:, k * 512:(k + 1) * 512], ps2[k])
            return Rrep

        def head_av(h, Vh8, Eh8, Rrep):
            """o^T*32 = (V*32)^T E^T / sums, fp8. Drain: DVE TT mult."""
            oT8 = [hp.tile([128, 2, S], fp8, name=f"oT{p}",
                           tag=f"oT{p}_{h % 2}") for p in range(2)]
            for ec in range(4):
                ps2 = [psum_mm.tile([128, 512], f32, name="po", tag="pmm")
                       for _ in range(2)]
                dr_accum(
                    ps2,
                    lambda tp: Vh8[tp][:, :, ec * 128:(ec + 1) * 128],
                    lambda tp, n4: Eh8[tp][:, :, n4 * 256:(n4 + 1) * 256], 4)
                for k in range(2):
                    nc.vector.tensor_tensor(
                        oT8[ec // 2][:, ec % 2, k * 512:(k + 1) * 512],
                        ps2[k], Rrep[:, k * 512:(k + 1) * 512], op=ALU.mult)
            return oT8

        def head_lv(h, oT8, lvw_t, last):
            """attn_acc += (o^T*32)^T @ (lvw*32); scale folded at the end."""
            for t in range(8):
                pl = psum_mm.tile([128, E], f32, name="pl", tag="pmm")
                for c in range(2):
                    for p in range(2):
                        nc.tensor.matmul(
                            pl[:, c * 256:(c + 1) * 256],
                            oT8[p][:, :, t * 128:(t + 1) * 128],
                            lvw_t[p][:, :, c * 256:(c + 1) * 256],
                            start=(p == 0 and c == 0), stop=(p == 1),
                            perf_mode=DR)
                if h == 0:
                    nc.vector.tensor_copy(attn_acc[t], pl)
                else:
                    nc.vector.tensor_add(attn_acc[t], attn_acc[t], pl)
                if last:
                    ln2_tile(t)

        # Pipelined loop. PE stream per iteration:
        #   A(h) -> V(h) -> scores(h) -> sums(h-1) -> AV(h-1) -> lv(h-1)
        prev = None
        for h in range(H):
            m_t, wv_t, lvw_t = PRE0 if h == 0 else emit_head_dma(h)
            AT8 = head_A(h, m_t)
            Vh8 = head_v(h, wv_t)
            Eh8 = head_scores(h, AT8)
            if prev is not None:
                ph, pV, pE, plvw = prev
                Rrep = head_sums(ph, pE)
                oT8 = head_av(ph, pV, pE, Rrep)
                head_lv(ph, oT8, plvw, last=False)
            if h == 0:
                emit_adaln_a1_seed()
            elif h == 2:
                emit_adaln_ln2()
            elif h >= 4:
                emit_ffw_quarter(h - 4)
            prev = (h, Vh8, Eh8, lvw_t)
        ph, pV, pE, plvw = prev
        Rrep = head_sums(ph, pE)
        oT8 = head_av(ph, pV, pE, Rrep)
        head_lv(ph, oT8, plvw, last=True)
        hp_ctx.close()

        # keep the PE clock-gate open across the LN2 boundary
        for i in range(8):
            warm2 = psum_tp.tile([128, 256], bf16, name="warm2", tag="ptp")
            nc.tensor.matmul(warm2[:, 0:128], ident, ident,
                             start=True, stop=True, is_transpose=True)

        # ---- Phase 5: FFN ----
        emit_f1(0)
        emit_f1(1)
        for t in range(8):
            pz = psum_mm.tile([128, E], f32, name="pz", tag="pmm")
            for kc in range(16):
                nc.tensor.matmul(
                    pz, hT[kc][:, t * 128:(t + 1) * 128], f2w_t[kc],
                    start=(kc == 0), stop=(kc == 15))
            q1 = work.tile([128, E], f32, name="q1", tag="wbig")
            nc.vector.tensor_add(q1, pz, MOD["F2B"])
            nc.vector.tensor_mul(q1, q1, MOD["A2"])
            ot = work.tile([128, E], f32, name="ot", tag="wbig")
            nc.vector.tensor_add(ot, q1, y2[t])
            nc.sync.dma_start(out_d[t * 128:(t + 1) * 128, :], ot)

    nc.compile()
    return nc


def _get_program():
    if "nc" not in _CACHE:
        _CACHE["nc"] = _build()
    return _CACHE["nc"]


def _pair4(w):
    """[512, C] -> [2, 128, 2, C]: chunk c=2p+j of the contraction dim goes
    to plane j of pair p (DoubleRow layout)."""
    C = w.shape[1]
    return np.ascontiguousarray(
        w.reshape(2, 2, 128, C).transpose(0, 2, 1, 3))


def kernel(**inputs) -> np.ndarray:
    import ml_dtypes
    from concourse.bass_utils import run_bass_kernel_spmd

    fp8 = ml_dtypes.float8_e4m3
    bf16 = ml_dtypes.bfloat16
    ins = {k: np.asarray(v, dtype=np.float32) for k, v in inputs.items()}
    nc = _get_program()

    # host-side fp8 weight prep (shared across cores)
    wq, wk, wv, lvw = ins["wq"], ins["wk"], ins["wv"], ins["lvw"]
    assert np.abs(ins["bq"]).max() == 0 and np.abs(ins["bk"]).max() == 0, (
        "fast path assumes zero attention q/k biases (true for this model)")
    m8 = np.stack([
        _pair4((wq[:, h * E:(h + 1) * E] @ wk[:, h * E:(h + 1) * E].T) * MS)
        for h in range(H)])
    wv8 = np.stack([_pair4(wv[:, h * E:(h + 1) * E] * WS) for h in range(H)])
    lvw8 = np.stack([_pair4(lvw[h * E:(h + 1) * E, :] * WS)
                     for h in range(H)])
    # V bias folded through the (linear) attention average into lvb
    lvb_eff = ins["lvb"].reshape(E) + ins["bv"].reshape(HE) @ lvw

    common = {
        "ln1g": ins["ln1g"].reshape(1, E), "ln1b": ins["ln1b"].reshape(1, E),
        "ln2g": ins["ln2g"].reshape(1, E), "ln2b": ins["ln2b"].reshape(1, E),
        "m8": m8.astype(fp8), "wv8": wv8.astype(fp8),
        "lvw8": lvw8.astype(fp8),
        "f1w16": ins["f1w"].astype(bf16), "f2w16": ins["f2w"].astype(bf16),
        "lvb": lvb_eff.reshape(1, E).astype(np.float32),
        "f1b": ins["f1b"].reshape(1, FF),
        "f2b": ins["f2b"].reshape(1, E),
        "ident": np.eye(128, dtype=np.float32).astype(bf16),
    }
    for nm in ["g1", "be1", "a1", "g2", "be2", "a2"]:
        common[f"{nm}w"] = ins[f"{nm}w"].astype(bf16)
        common[f"{nm}b"] = ins[f"{nm}b"].reshape(1, E)

    in_maps = []
    for b in range(B):
        m = dict(common)
        m["x"] = ins["x"][b]
        m["cond"] = ins["cond"][b].reshape(E, 1)
        in_maps.append(m)

    res = run_bass_kernel_spmd(nc, in_maps, list(range(N_CORES)),
                               trace=TRACE, tmpdir=TRACE_DIR)
    _CACHE["last_result"] = res
    out = np.stack([res.results[b]["out"] for b in range(B)], axis=0)
    return out
